# revision 23
# baseline (speedup 1.0000x reference)
"""BankedLinear (MoE-style banked linear) Trainium2 kernel.

Reference computation (per token t, with k=2 selected banks):
    out[t] = sum_k prob[t,k] * (x[t] @ W[sel[t,k]] + bias[sel[t,k]])

Strategy (expert-parallel over 8 NeuronCores):
  - Core c owns banks [8c, 8c+8).  Its weight slab is the dominant HBM
    traffic; each bank is read exactly once system-wide.
  - Host routes token-bank pairs to cores by selected bank, pre-scales each
    gathered token row by its probability, transposes to [in_feature, slot],
    and pads to CAP=32 slots per bank.
  - Precision: everything runs in a single bf16 matmul term (x_bf16 @ W_bf16
    accumulated in fp32 PSUM, ~2e-3 rel error, well under the 2e-2 gate).
    This halves weight DMA bytes and cuts PE work 3x vs an fp32-faithful
    hi/lo split.
  - Weights stream as a few ~1MB HWDGE DMAs on the sync ring (big transfers
    amortize the ~600ns per-DMA issue cost and use 8KB/partition descriptor
    lines); the last pairs are split by contraction chunk so the final
    matmuls overlap the stream tail.  x and y ride the scalar (Activation)
    HWDGE ring so they never queue behind the weight stream.
  - PE: banks run 2 pairs per PSUM bank with 4-way column tiling
    (tile_position col groups 0-3), so up to 4 banks' matmuls stream
    concurrently through the 128x128 array.  A single start=True matmul
    clears each PSUM bank; later matmuls overwrite-where-clear /
    accumulate-where-set (per-element has_written semantics).
  - Bias is folded in on the host (one gather + multiply-add over 1024
    pairs); host scatter-adds the per-pair device results into the output.

Fixed shapes: B=2, T=256, K=2, IN=OUT=512, NB=64 banks, 8 cores.
Capacity: 32 slots/bank (binomial mean 16, sd ~4; overflow pairs -- none for
realistic routing -- are handled exactly on the host as a fallback).
"""

import numpy as np
from contextlib import ExitStack

B, T, KSEL = 2, 256, 2
IN, OUT, NB = 512, 512, 64
NCORES = 8
BPC = NB // NCORES          # banks per core = 8
CAP = 32                    # padded token slots per bank
SLOTS = BPC * CAP           # 256 dispatch rows per core
PCHUNK = 128                # contraction chunk (SBUF partition dim)
KC = IN // PCHUNK           # 4 contraction chunks
NPAIR = BPC // 2            # 4 bank pairs
GROUPS = 2                  # PSUM banks / output row groups of 128

# weight DMA chunking per pair (kc ranges); later pairs split finer so the
# final matmuls overlap the end of the weight stream
WCHUNKS = {
    0: [(0, 4)],
    1: [(0, 4)],
    2: [(0, 2), (2, 4)],
    3: [(0, 1), (1, 2), (2, 3), (3, 4)],
}

_cache = {}


def _build_nc():
    """Build the Bass/Tile program (one SPMD NeuronCore program)."""
    import concourse.tile as tile
    import concourse.mybir as mybir
    from concourse import bacc
    from concourse.tile import add_dep_helper

    f32 = mybir.dt.float32
    bf16 = mybir.dt.bfloat16
    nc = bacc.Bacc("TRN2", target_bir_lowering=False, debug=False,
                   num_devices=NCORES)
    # host-pre-swizzled layouts: partition dim first, contiguous free dim
    # xt free index: (kc, slot)
    xt = nc.dram_tensor("xt", [PCHUNK, KC * SLOTS], bf16,
                        kind="ExternalInput").ap()
    # w free index: (pair, kc, q, out) so both pair-sized and kc-sized DMA
    # slices are per-partition contiguous
    w = nc.dram_tensor("w", [PCHUNK, BPC * KC * OUT], bf16,
                       kind="ExternalInput").ap()
    # y free index: (group, out); row = slot within group
    y = nc.dram_tensor("y", [PCHUNK, GROUPS * OUT], f32,
                       kind="ExternalOutput").ap()

    def chain(dep_chain, binst, reason):
        # pin scheduler order: binst depends on the previous link
        if dep_chain:
            add_dep_helper(binst.ins, dep_chain[-1].ins, sync=False,
                           reason=reason)
        dep_chain.append(binst)

    with tile.TileContext(nc) as tc:
        with ExitStack() as ctx:
            xpool = ctx.enter_context(tc.tile_pool(name="xp", bufs=1))
            wpool = ctx.enter_context(tc.tile_pool(name="wp", bufs=1))
            ypool = ctx.enter_context(tc.tile_pool(name="yp", bufs=GROUPS))
            pspool = ctx.enter_context(
                tc.tile_pool(name="ps", bufs=BPC, space="PSUM"))

            xt_sb = xpool.tile([PCHUNK, KC * SLOTS], bf16, tag="x")
            ysbs = [ypool.tile([PCHUNK, OUT], f32, tag="y", name=f"ysb{g}")
                    for g in range(GROUPS)]
            # one PSUM bank per expert bank (8 chains, 8 banks): each
            # accumulation chain starts with its own start=True clear in its
            # own bank, so no chain can disturb -- or inherit stale
            # has_written state from -- another (the clear is bank-wide)
            pss = [pspool.tile([PCHUNK, OUT], f32, tag="ps", name=f"ps{j}")
                   for j in range(BPC)]

            sq = []   # scalar HWDGE ring: xt load, then y stores
            wq = []   # sync HWDGE ring: weight stream in compute order
            mq = []   # PE chain: per-pair compute order

            # xt on the scalar ring: it interleaves with the weight stream
            # (sharing the 16 SDMA engines) and lands by ~pair-1 time, which
            # the chunked matmul pipeline absorbs; prepending it to the sync
            # ring instead would push the whole weight stream ~0.8us later.
            chain(sq, nc.scalar.dma_start(xt_sb[:], xt[:]), "xt first")

            wtiles = {}
            for p in range(NPAIR):
                wtiles[p] = []
                for (k0, k1) in WCHUNKS[p]:
                    t = wpool.tile([PCHUNK, (k1 - k0) * 2 * OUT], bf16,
                                   name=f"w{p}_{k0}")
                    src = w[:, (p * KC + k0) * 2 * OUT:
                            (p * KC + k1) * 2 * OUT]
                    chain(wq, nc.sync.dma_start(t[:], src), "w ring order")
                    wtiles[p].append((k0, k1, t))

            for p in range(NPAIR):
                g, h = divmod(p, 2)
                for kc in range(KC):
                    for (k0, k1, t) in wtiles[p]:
                        if k0 <= kc < k1:
                            break
                    for q in range(2):
                        j = 2 * p + q
                        c = 2 * h + q       # PE column group 0..3
                        rhs = t[:, ((kc - k0) * 2 + q) * OUT:
                                ((kc - k0) * 2 + q + 1) * OUT]
                        lhsT = xt_sb[:, kc * SLOTS + j * CAP:
                                     kc * SLOTS + (j + 1) * CAP]
                        outap = pss[j][32 * c:32 * (c + 1), :]
                        mm = nc.tensor.matmul(
                            outap, lhsT, rhs,
                            start=(kc == 0),
                            stop=(kc == KC - 1),
                            tile_position=(0, 32 * c),
                            skip_group_check=True)
                        # chain EVERY matmul to pin PE issue order (keeps the
                        # kc-major interleave the pipeline was designed for)
                        chain(mq, mm, "PE issue order")
                # copy each bank's slice out of its own psum bank; the odd
                # bank goes via the scalar (Activation) engine so the last
                # pair's two copies run in parallel on different banks
                r0 = slice(64 * h, 64 * h + 32)
                r1 = slice(64 * h + 32, 64 * h + 64)
                nc.vector.tensor_copy(ysbs[g][r0, :], pss[2 * p][r0, :])
                nc.scalar.copy(ysbs[g][r1, :], pss[2 * p + 1][r1, :])
                # wave A: one 256KB store; wave B: store rows [0:64)
                # early (after pair 2) and rows [64:128) at the end
                if p == 1:
                    chain(sq, nc.scalar.dma_start(
                        y[:, g * OUT:(g + 1) * OUT], ysbs[g][:]),
                        "yA after copy")
                elif p == 2:
                    chain(sq, nc.scalar.dma_start(
                        y[0:64, g * OUT:(g + 1) * OUT], ysbs[g][0:64, :]),
                        "yB0 after copy")
                elif p == 3:
                    chain(sq, nc.scalar.dma_start(
                        y[64:128, g * OUT:(g + 1) * OUT], ysbs[g][64:128, :]),
                        "yB1 after copy")
    nc.compile()
    return nc


def _get_nc():
    if "nc" not in _cache:
        _cache["nc"] = _build_nc()
    return _cache["nc"]


def _bf16(a32):
    import ml_dtypes
    return a32.astype(ml_dtypes.bfloat16)


def _swizzle_x(xt):
    """[IN, SLOTS] -> [128, KC*SLOTS] with free index (kc, slot)."""
    return np.ascontiguousarray(
        xt.reshape(KC, PCHUNK, SLOTS).transpose(1, 0, 2).reshape(
            PCHUNK, KC * SLOTS))


def _swizzle_w(wb):
    """[BPC, IN, OUT] bf16 -> [128, NPAIR*KC*2*OUT], free (pair, kc, q, out)."""
    # (pair, q, kc, row, out) -> (row, pair, kc, q, out)
    return np.ascontiguousarray(
        wb.reshape(NPAIR, 2, KC, PCHUNK, OUT).transpose(3, 0, 2, 1, 4)
        .reshape(PCHUNK, NPAIR * KC * 2 * OUT))


def _route(X, sel, prob):
    """Group token-bank pairs by bank, build per-core dispatch arrays.

    Returns (slot_tok [NCORES,SLOTS] int64 (-1=pad), slot_p, overflow list
    of (token, bank, prob))."""
    NT = X.shape[0]
    pair_tok = np.repeat(np.arange(NT, dtype=np.int64), KSEL)
    pair_bank = sel.reshape(-1)
    pair_p = prob.reshape(-1)

    order = np.argsort(pair_bank, kind="stable")
    counts = np.bincount(pair_bank, minlength=NB)
    starts = np.concatenate(([0], np.cumsum(counts)))

    slot_tok = np.full((NCORES, SLOTS), -1, dtype=np.int64)
    slot_p = np.zeros((NCORES, SLOTS), dtype=np.float32)
    overflow = []
    for b in range(NB):
        c, j = divmod(b, BPC)
        s0, s1 = starts[b], starts[b + 1]
        take = min(s1 - s0, CAP)
        idx = order[s0:s0 + take]
        slot_tok[c, j * CAP: j * CAP + take] = pair_tok[idx]
        slot_p[c, j * CAP: j * CAP + take] = pair_p[idx]
        for i in order[s0 + take:s1]:
            overflow.append((int(pair_tok[i]), b, float(pair_p[i])))
    return slot_tok, slot_p, overflow


def _combine(ys, slot_tok, X, sel, prob, weights, bias, overflow):
    NT = X.shape[0]
    out = np.zeros((NT, OUT), dtype=np.float32)
    for c in range(NCORES):
        tok = slot_tok[c]
        valid = tok >= 0
        np.add.at(out, tok[valid], ys[c][valid])
    # bias term for every pair (device computes x @ W only)
    for k in range(KSEL):
        out += prob[:, k, None] * bias[sel[:, k]]
    # exact host fallback for capacity-overflow pairs (expected: none)
    for t, b, p in overflow:
        out[t] += p * (X[t] @ weights[b])
    return out


def _run_device(in_maps, trace=False, **kwargs):
    from concourse.bass_utils import run_bass_kernel_spmd
    return run_bass_kernel_spmd(_get_nc(), in_maps,
                                core_ids=list(range(NCORES)),
                                trace=trace, **kwargs)


def kernel(_trace=False, _bass_results=None, **inputs):
    tensor = np.asarray(inputs["tensor"], dtype=np.float32)
    sel = np.asarray(inputs["bank_selections"]).astype(np.int64)
    prob = np.asarray(inputs["bank_probabilities"], dtype=np.float32)
    weights = np.asarray(inputs["weights"], dtype=np.float32)
    bias = np.asarray(inputs["bias"], dtype=np.float32)

    NT = tensor.shape[0] * tensor.shape[1]
    X = tensor.reshape(NT, IN)
    sel2 = sel.reshape(NT, KSEL)
    prob2 = prob.reshape(NT, KSEL)

    slot_tok, slot_p, overflow = _route(X, sel2, prob2)

    in_maps = []
    for c in range(NCORES):
        tok = slot_tok[c]
        rows = X[np.where(tok >= 0, tok, 0)] * slot_p[c][:, None]
        xt = np.ascontiguousarray(rows.T)              # [IN, SLOTS] fp32
        wb = _bf16(weights[c * BPC:(c + 1) * BPC])     # (8, 512, 512) bf16
        in_maps.append({
            "xt": _swizzle_x(_bf16(xt)),
            "w": _swizzle_w(wb),
        })

    res = _run_device(in_maps, trace=_trace)
    if _bass_results is not None:
        _bass_results.append(res)
    # y: [128, GROUPS*OUT] f32; row r of group g is slot g*128 + r
    ys = []
    for c in range(NCORES):
        yflat = np.asarray(res.results[c]["y"]).astype(np.float32)
        ys.append(np.concatenate(
            [yflat[:, g * OUT:(g + 1) * OUT] for g in range(GROUPS)],
            axis=0))

    out = _combine(ys, slot_tok, X, sel2, prob2, weights, bias, overflow)
    return out.reshape(tensor.shape[0], tensor.shape[1], OUT)


# revision 24
# speedup vs baseline: 1.0797x; 1.0797x over previous
"""BankedLinear (MoE-style banked linear) Trainium2 kernel.

Reference computation (per token t, with k=2 selected banks):
    out[t] = sum_k prob[t,k] * (x[t] @ W[sel[t,k]] + bias[sel[t,k]])

Strategy (expert-parallel over 8 NeuronCores):
  - Core c owns banks [8c, 8c+8).  Its weight slab is the dominant HBM
    traffic; each bank is read exactly once system-wide.
  - Host routes token-bank pairs to cores by selected bank, pre-scales each
    gathered token row by its probability, transposes to [in_feature, slot],
    and pads to CAP=32 slots per bank.
  - Precision: everything runs in a single bf16 matmul term (x_bf16 @ W_bf16
    accumulated in fp32 PSUM, ~2e-3 rel error, well under the 2e-2 gate).
    This halves weight DMA bytes and cuts PE work 3x vs an fp32-faithful
    hi/lo split.
  - Weights stream as a few ~1MB HWDGE DMAs on the sync ring (big transfers
    amortize the ~600ns per-DMA issue cost and use 8KB/partition descriptor
    lines); the last pairs are split by contraction chunk so the final
    matmuls overlap the stream tail.  x and y ride the scalar (Activation)
    HWDGE ring so they never queue behind the weight stream.
  - PE: banks run 2 pairs per PSUM bank with 4-way column tiling
    (tile_position col groups 0-3), so up to 4 banks' matmuls stream
    concurrently through the 128x128 array.  A single start=True matmul
    clears each PSUM bank; later matmuls overwrite-where-clear /
    accumulate-where-set (per-element has_written semantics).
  - Bias is folded in on the host (one gather + multiply-add over 1024
    pairs); host scatter-adds the per-pair device results into the output.

Fixed shapes: B=2, T=256, K=2, IN=OUT=512, NB=64 banks, 8 cores.
Capacity: 32 slots/bank (binomial mean 16, sd ~4; overflow pairs -- none for
realistic routing -- are handled exactly on the host as a fallback).
"""

import numpy as np
from contextlib import ExitStack

B, T, KSEL = 2, 256, 2
IN, OUT, NB = 512, 512, 64
NCORES = 8
BPC = NB // NCORES          # banks per core = 8
CAP = 32                    # padded token slots per bank
SLOTS = BPC * CAP           # 256 dispatch rows per core
PCHUNK = 128                # contraction chunk (SBUF partition dim)
KC = IN // PCHUNK           # 4 contraction chunks
NPAIR = BPC // 2            # 4 bank pairs
GROUPS = 2                  # PSUM banks / output row groups of 128

# weight DMA chunking per pair (kc ranges); later pairs split finer so the
# final matmuls overlap the end of the weight stream
WCHUNKS = {
    0: [(0, 4)],
    1: [(0, 4)],
    2: [(0, 2), (2, 4)],
    3: [(0, 1), (1, 2), (2, 3), (3, 4)],
}

_cache = {}


def _build_nc():
    """Build the Bass/Tile program (one SPMD NeuronCore program)."""
    import concourse.tile as tile
    import concourse.mybir as mybir
    from concourse import bacc
    from concourse.tile import add_dep_helper

    f32 = mybir.dt.float32
    bf16 = mybir.dt.bfloat16
    nc = bacc.Bacc("TRN2", target_bir_lowering=False, debug=False,
                   num_devices=1)
    # host-pre-swizzled layouts: partition dim first, contiguous free dim
    # xt free index: (kc, slot)
    xt = nc.dram_tensor("xt", [PCHUNK, KC * SLOTS], bf16,
                        kind="ExternalInput").ap()
    # w free index: (pair, kc, q, out) so both pair-sized and kc-sized DMA
    # slices are per-partition contiguous
    w = nc.dram_tensor("w", [PCHUNK, BPC * KC * OUT], bf16,
                       kind="ExternalInput").ap()
    # y free index: (group, out); row = slot within group
    y = nc.dram_tensor("y", [PCHUNK, GROUPS * OUT], f32,
                       kind="ExternalOutput").ap()

    def chain(dep_chain, binst, reason):
        # pin scheduler order: binst depends on the previous link
        if dep_chain:
            add_dep_helper(binst.ins, dep_chain[-1].ins, sync=False,
                           reason=reason)
        dep_chain.append(binst)

    with tile.TileContext(nc) as tc:
        with ExitStack() as ctx:
            xpool = ctx.enter_context(tc.tile_pool(name="xp", bufs=1))
            wpool = ctx.enter_context(tc.tile_pool(name="wp", bufs=1))
            ypool = ctx.enter_context(tc.tile_pool(name="yp", bufs=GROUPS))
            pspool = ctx.enter_context(
                tc.tile_pool(name="ps", bufs=BPC, space="PSUM"))

            xt_sb = xpool.tile([PCHUNK, KC * SLOTS], bf16, tag="x")
            ysbs = [ypool.tile([PCHUNK, OUT], f32, tag="y", name=f"ysb{g}")
                    for g in range(GROUPS)]
            # one PSUM bank per expert bank (8 chains, 8 banks): each
            # accumulation chain starts with its own start=True clear in its
            # own bank, so no chain can disturb -- or inherit stale
            # has_written state from -- another (the clear is bank-wide)
            pss = [pspool.tile([PCHUNK, OUT], f32, tag="ps", name=f"ps{j}")
                   for j in range(BPC)]

            sq = []   # scalar HWDGE ring: xt load, then y stores
            wq = []   # sync HWDGE ring: weight stream in compute order
            mq = []   # PE chain: per-pair compute order

            # xt on the scalar ring: it interleaves with the weight stream
            # (sharing the 16 SDMA engines) and lands by ~pair-1 time, which
            # the chunked matmul pipeline absorbs; prepending it to the sync
            # ring instead would push the whole weight stream ~0.8us later.
            chain(sq, nc.scalar.dma_start(xt_sb[:], xt[:]), "xt first")

            wtiles = {}
            for p in range(NPAIR):
                wtiles[p] = []
                for (k0, k1) in WCHUNKS[p]:
                    t = wpool.tile([PCHUNK, (k1 - k0) * 2 * OUT], bf16,
                                   name=f"w{p}_{k0}")
                    src = w[:, (p * KC + k0) * 2 * OUT:
                            (p * KC + k1) * 2 * OUT]
                    chain(wq, nc.sync.dma_start(t[:], src), "w ring order")
                    wtiles[p].append((k0, k1, t))

            for p in range(NPAIR):
                g, h = divmod(p, 2)
                for kc in range(KC):
                    for (k0, k1, t) in wtiles[p]:
                        if k0 <= kc < k1:
                            break
                    for q in range(2):
                        j = 2 * p + q
                        c = 2 * h + q       # PE column group 0..3
                        rhs = t[:, ((kc - k0) * 2 + q) * OUT:
                                ((kc - k0) * 2 + q + 1) * OUT]
                        lhsT = xt_sb[:, kc * SLOTS + j * CAP:
                                     kc * SLOTS + (j + 1) * CAP]
                        outap = pss[j][32 * c:32 * (c + 1), :]
                        mm = nc.tensor.matmul(
                            outap, lhsT, rhs,
                            start=(kc == 0),
                            stop=(kc == KC - 1),
                            tile_position=(0, 32 * c),
                            skip_group_check=True)
                        # chain EVERY matmul to pin PE issue order (keeps the
                        # kc-major interleave the pipeline was designed for)
                        chain(mq, mm, "PE issue order")
                # copy each bank's slice out of its own psum bank; the odd
                # bank goes via the scalar (Activation) engine so the last
                # pair's two copies run in parallel on different banks
                r0 = slice(64 * h, 64 * h + 32)
                r1 = slice(64 * h + 32, 64 * h + 64)
                nc.vector.tensor_copy(ysbs[g][r0, :], pss[2 * p][r0, :])
                nc.scalar.copy(ysbs[g][r1, :], pss[2 * p + 1][r1, :])
                # wave A: one 256KB store; wave B: store rows [0:64)
                # early (after pair 2) and rows [64:128) at the end
                if p == 1:
                    chain(sq, nc.scalar.dma_start(
                        y[:, g * OUT:(g + 1) * OUT], ysbs[g][:]),
                        "yA after copy")
                elif p == 2:
                    chain(sq, nc.scalar.dma_start(
                        y[0:64, g * OUT:(g + 1) * OUT], ysbs[g][0:64, :]),
                        "yB0 after copy")
                elif p == 3:
                    chain(sq, nc.scalar.dma_start(
                        y[64:128, g * OUT:(g + 1) * OUT], ysbs[g][64:128, :]),
                        "yB1 after copy")
    nc.compile()
    return nc


def _get_nc():
    if "nc" not in _cache:
        _cache["nc"] = _build_nc()
    return _cache["nc"]


def _bf16(a32):
    import ml_dtypes
    return a32.astype(ml_dtypes.bfloat16)


def _swizzle_x(xt):
    """[IN, SLOTS] -> [128, KC*SLOTS] with free index (kc, slot)."""
    return np.ascontiguousarray(
        xt.reshape(KC, PCHUNK, SLOTS).transpose(1, 0, 2).reshape(
            PCHUNK, KC * SLOTS))


def _swizzle_w(wb):
    """[BPC, IN, OUT] bf16 -> [128, NPAIR*KC*2*OUT], free (pair, kc, q, out)."""
    # (pair, q, kc, row, out) -> (row, pair, kc, q, out)
    return np.ascontiguousarray(
        wb.reshape(NPAIR, 2, KC, PCHUNK, OUT).transpose(3, 0, 2, 1, 4)
        .reshape(PCHUNK, NPAIR * KC * 2 * OUT))


def _route(X, sel, prob):
    """Group token-bank pairs by bank, build per-core dispatch arrays.

    Returns (slot_tok [NCORES,SLOTS] int64 (-1=pad), slot_p, overflow list
    of (token, bank, prob))."""
    NT = X.shape[0]
    pair_tok = np.repeat(np.arange(NT, dtype=np.int64), KSEL)
    pair_bank = sel.reshape(-1)
    pair_p = prob.reshape(-1)

    order = np.argsort(pair_bank, kind="stable")
    counts = np.bincount(pair_bank, minlength=NB)
    starts = np.concatenate(([0], np.cumsum(counts)))

    slot_tok = np.full((NCORES, SLOTS), -1, dtype=np.int64)
    slot_p = np.zeros((NCORES, SLOTS), dtype=np.float32)
    overflow = []
    for b in range(NB):
        c, j = divmod(b, BPC)
        s0, s1 = starts[b], starts[b + 1]
        take = min(s1 - s0, CAP)
        idx = order[s0:s0 + take]
        slot_tok[c, j * CAP: j * CAP + take] = pair_tok[idx]
        slot_p[c, j * CAP: j * CAP + take] = pair_p[idx]
        for i in order[s0 + take:s1]:
            overflow.append((int(pair_tok[i]), b, float(pair_p[i])))
    return slot_tok, slot_p, overflow


def _combine(ys, slot_tok, X, sel, prob, weights, bias, overflow):
    NT = X.shape[0]
    out = np.zeros((NT, OUT), dtype=np.float32)
    for c in range(NCORES):
        tok = slot_tok[c]
        valid = tok >= 0
        np.add.at(out, tok[valid], ys[c][valid])
    # bias term for every pair (device computes x @ W only)
    for k in range(KSEL):
        out += prob[:, k, None] * bias[sel[:, k]]
    # exact host fallback for capacity-overflow pairs (expected: none)
    for t, b, p in overflow:
        out[t] += p * (X[t] @ weights[b])
    return out


def _run_device(in_maps, trace=False, **kwargs):
    from concourse.bass_utils import run_bass_kernel_spmd
    return run_bass_kernel_spmd(_get_nc(), in_maps,
                                core_ids=list(range(NCORES)),
                                trace=trace, **kwargs)


def kernel(_trace=False, _bass_results=None, **inputs):
    tensor = np.asarray(inputs["tensor"], dtype=np.float32)
    sel = np.asarray(inputs["bank_selections"]).astype(np.int64)
    prob = np.asarray(inputs["bank_probabilities"], dtype=np.float32)
    weights = np.asarray(inputs["weights"], dtype=np.float32)
    bias = np.asarray(inputs["bias"], dtype=np.float32)

    NT = tensor.shape[0] * tensor.shape[1]
    X = tensor.reshape(NT, IN)
    sel2 = sel.reshape(NT, KSEL)
    prob2 = prob.reshape(NT, KSEL)

    slot_tok, slot_p, overflow = _route(X, sel2, prob2)

    in_maps = []
    for c in range(NCORES):
        tok = slot_tok[c]
        rows = X[np.where(tok >= 0, tok, 0)] * slot_p[c][:, None]
        xt = np.ascontiguousarray(rows.T)              # [IN, SLOTS] fp32
        wb = _bf16(weights[c * BPC:(c + 1) * BPC])     # (8, 512, 512) bf16
        in_maps.append({
            "xt": _swizzle_x(_bf16(xt)),
            "w": _swizzle_w(wb),
        })

    res = _run_device(in_maps, trace=_trace)
    if _bass_results is not None:
        _bass_results.append(res)
    # y: [128, GROUPS*OUT] f32; row r of group g is slot g*128 + r
    ys = []
    for c in range(NCORES):
        yflat = np.asarray(res.results[c]["y"]).astype(np.float32)
        ys.append(np.concatenate(
            [yflat[:, g * OUT:(g + 1) * OUT] for g in range(GROUPS)],
            axis=0))

    out = _combine(ys, slot_tok, X, sel2, prob2, weights, bias, overflow)
    return out.reshape(tensor.shape[0], tensor.shape[1], OUT)


# revision 25
# speedup vs baseline: 1.0896x; 1.0092x over previous
"""BankedLinear (MoE-style banked linear) Trainium2 kernel.

Reference computation (per token t, with k=2 selected banks):
    out[t] = sum_k prob[t,k] * (x[t] @ W[sel[t,k]] + bias[sel[t,k]])

Strategy (expert-parallel over 8 NeuronCores):
  - Core c owns banks [8c, 8c+8).  Its weight slab is the dominant HBM
    traffic; each bank is read exactly once system-wide.
  - Host routes token-bank pairs to cores by selected bank, pre-scales each
    gathered token row by its probability, transposes to [in_feature, slot],
    and pads to CAP=32 slots per bank.
  - Precision: everything runs in a single bf16 matmul term (x_bf16 @ W_bf16
    accumulated in fp32 PSUM, ~2e-3 rel error, well under the 2e-2 gate).
    This halves weight DMA bytes and cuts PE work 3x vs an fp32-faithful
    hi/lo split.
  - Weights stream as a few ~1MB HWDGE DMAs on the sync ring (big transfers
    amortize the ~600ns per-DMA issue cost and use 8KB/partition descriptor
    lines, hitting ~360GB/s -- the per-core HBM wall); the last pairs are
    split by contraction chunk so the final matmuls overlap the stream
    tail.  x and y ride the scalar (Activation) HWDGE ring so they never
    queue behind the weight stream.
  - PE: bank pairs run with 4-way column tiling (tile_position col groups
    0-3 alternate between pairs), so up to 4 banks' matmuls stream
    concurrently through the 128x128 array.  Each bank's accumulation
    chain owns a full PSUM bank and opens with its own start=True matmul:
    the start=True has_written clear is bank-wide, so chains must never
    share a bank (sharing makes results depend on stale PSUM state across
    executions).  All matmuls are dependency-chained so the scheduler
    cannot reorder a chain's clear after a sibling's first write.
  - Per-pair PSUM->SBUF copies split across DVE (even bank) and ACT (odd
    bank) so the exposed tail copy runs once, not twice; wave-B output
    stores in two chunks so only the last 128KB store sits on the tail.
  - Bias is folded in on the host (one gather + multiply-add over 1024
    pairs); host scatter-adds the per-pair device results into the output.

Fixed shapes: B=2, T=256, K=2, IN=OUT=512, NB=64 banks, 8 cores.
Capacity: 32 slots/bank (binomial mean 16, sd ~4; overflow pairs -- none for
realistic routing -- are handled exactly on the host as a fallback).
"""

import numpy as np
from contextlib import ExitStack

B, T, KSEL = 2, 256, 2
IN, OUT, NB = 512, 512, 64
NCORES = 8
BPC = NB // NCORES          # banks per core = 8
CAP = 32                    # padded token slots per bank
SLOTS = BPC * CAP           # 256 dispatch rows per core
PCHUNK = 128                # contraction chunk (SBUF partition dim)
KC = IN // PCHUNK           # 4 contraction chunks
NPAIR = BPC // 2            # 4 bank pairs
GROUPS = 2                  # PSUM banks / output row groups of 128

# weight DMA chunking per pair (kc ranges); later pairs split finer so the
# final matmuls overlap the end of the weight stream
WCHUNKS = {
    0: [(0, 4)],
    1: [(0, 4)],
    2: [(0, 2), (2, 4)],
    3: [(0, 1), (1, 2), (2, 3), (3, 4)],
}

_cache = {}


def _build_nc():
    """Build the Bass/Tile program (one SPMD NeuronCore program)."""
    import concourse.tile as tile
    import concourse.mybir as mybir
    from concourse import bacc
    from concourse.tile import add_dep_helper

    f32 = mybir.dt.float32
    bf16 = mybir.dt.bfloat16
    nc = bacc.Bacc("TRN2", target_bir_lowering=False, debug=False,
                   num_devices=1)
    # host-pre-swizzled layouts: partition dim first, contiguous free dim
    # xt free index: (kc, slot)
    xt = nc.dram_tensor("xt", [PCHUNK, KC * SLOTS], bf16,
                        kind="ExternalInput").ap()
    # w free index: (pair, kc, q, out) so both pair-sized and kc-sized DMA
    # slices are per-partition contiguous
    w = nc.dram_tensor("w", [PCHUNK, BPC * KC * OUT], bf16,
                       kind="ExternalInput").ap()
    # y free index: (group, out); row = slot within group
    y = nc.dram_tensor("y", [PCHUNK, GROUPS * OUT], f32,
                       kind="ExternalOutput").ap()

    def chain(dep_chain, binst, reason):
        # pin scheduler order: binst depends on the previous link
        if dep_chain:
            add_dep_helper(binst.ins, dep_chain[-1].ins, sync=False,
                           reason=reason)
        dep_chain.append(binst)

    with tile.TileContext(nc) as tc:
        with ExitStack() as ctx:
            xpool = ctx.enter_context(tc.tile_pool(name="xp", bufs=1))
            wpool = ctx.enter_context(tc.tile_pool(name="wp", bufs=1))
            ypool = ctx.enter_context(tc.tile_pool(name="yp", bufs=GROUPS))
            pspool = ctx.enter_context(
                tc.tile_pool(name="ps", bufs=BPC, space="PSUM"))

            xt_sb = xpool.tile([PCHUNK, KC * SLOTS], bf16, tag="x")
            ysbs = [ypool.tile([PCHUNK, OUT], f32, tag="y", name=f"ysb{g}")
                    for g in range(GROUPS)]
            # one PSUM bank per expert bank (8 chains, 8 banks): each
            # accumulation chain starts with its own start=True clear in its
            # own bank, so no chain can disturb -- or inherit stale
            # has_written state from -- another (the clear is bank-wide)
            pss = [pspool.tile([PCHUNK, OUT], f32, tag="ps", name=f"ps{j}")
                   for j in range(BPC)]

            sq = []   # scalar HWDGE ring: xt load, then y stores
            wq = []   # sync HWDGE ring: weight stream in compute order
            mq = []   # PE chain: per-pair compute order

            # xt on the scalar ring: it interleaves with the weight stream
            # (sharing the 16 SDMA engines) and lands by ~pair-1 time, which
            # the chunked matmul pipeline absorbs; prepending it to the sync
            # ring instead would push the whole weight stream ~0.8us later.
            chain(sq, nc.scalar.dma_start(xt_sb[:], xt[:]), "xt first")

            wtiles = {}
            for p in range(NPAIR):
                wtiles[p] = []
                for (k0, k1) in WCHUNKS[p]:
                    t = wpool.tile([PCHUNK, (k1 - k0) * 2 * OUT], bf16,
                                   name=f"w{p}_{k0}")
                    src = w[:, (p * KC + k0) * 2 * OUT:
                            (p * KC + k1) * 2 * OUT]
                    chain(wq, nc.sync.dma_start(t[:], src), "w ring order")
                    wtiles[p].append((k0, k1, t))

            for p in range(NPAIR):
                g, h = divmod(p, 2)
                for kc in range(KC):
                    for (k0, k1, t) in wtiles[p]:
                        if k0 <= kc < k1:
                            break
                    for q in range(2):
                        j = 2 * p + q
                        c = 2 * h + q       # PE column group 0..3
                        rhs = t[:, ((kc - k0) * 2 + q) * OUT:
                                ((kc - k0) * 2 + q + 1) * OUT]
                        lhsT = xt_sb[:, kc * SLOTS + j * CAP:
                                     kc * SLOTS + (j + 1) * CAP]
                        outap = pss[j][32 * c:32 * (c + 1), :]
                        mm = nc.tensor.matmul(
                            outap, lhsT, rhs,
                            start=(kc == 0),
                            stop=(kc == KC - 1),
                            tile_position=(0, 32 * c),
                            skip_group_check=True)
                        # chain EVERY matmul to pin PE issue order (keeps the
                        # kc-major interleave the pipeline was designed for)
                        chain(mq, mm, "PE issue order")
                # copy each bank's slice out of its own psum bank; the odd
                # bank goes via the scalar (Activation) engine so the last
                # pair's two copies run in parallel on different banks
                r0 = slice(64 * h, 64 * h + 32)
                r1 = slice(64 * h + 32, 64 * h + 64)
                nc.vector.tensor_copy(ysbs[g][r0, :], pss[2 * p][r0, :])
                nc.scalar.copy(ysbs[g][r1, :], pss[2 * p + 1][r1, :])
                # wave A: one 256KB store; wave B: store rows [0:64)
                # early (after pair 2) and rows [64:128) at the end
                if p == 1:
                    chain(sq, nc.scalar.dma_start(
                        y[:, g * OUT:(g + 1) * OUT], ysbs[g][:]),
                        "yA after copy")
                elif p == 2:
                    chain(sq, nc.scalar.dma_start(
                        y[0:64, g * OUT:(g + 1) * OUT], ysbs[g][0:64, :]),
                        "yB0 after copy")
                elif p == 3:
                    chain(sq, nc.scalar.dma_start(
                        y[64:128, g * OUT:(g + 1) * OUT], ysbs[g][64:128, :]),
                        "yB1 after copy")
    nc.compile()
    return nc


def _get_nc():
    if "nc" not in _cache:
        _cache["nc"] = _build_nc()
    return _cache["nc"]


def _bf16(a32):
    import ml_dtypes
    return a32.astype(ml_dtypes.bfloat16)


def _swizzle_x(xt):
    """[IN, SLOTS] -> [128, KC*SLOTS] with free index (kc, slot)."""
    return np.ascontiguousarray(
        xt.reshape(KC, PCHUNK, SLOTS).transpose(1, 0, 2).reshape(
            PCHUNK, KC * SLOTS))


def _swizzle_w(wb):
    """[BPC, IN, OUT] bf16 -> [128, NPAIR*KC*2*OUT], free (pair, kc, q, out)."""
    # (pair, q, kc, row, out) -> (row, pair, kc, q, out)
    return np.ascontiguousarray(
        wb.reshape(NPAIR, 2, KC, PCHUNK, OUT).transpose(3, 0, 2, 1, 4)
        .reshape(PCHUNK, NPAIR * KC * 2 * OUT))


def _route(X, sel, prob):
    """Group token-bank pairs by bank, build per-core dispatch arrays.

    Returns (slot_tok [NCORES,SLOTS] int64 (-1=pad), slot_p, overflow list
    of (token, bank, prob))."""
    NT = X.shape[0]
    pair_tok = np.repeat(np.arange(NT, dtype=np.int64), KSEL)
    pair_bank = sel.reshape(-1)
    pair_p = prob.reshape(-1)

    order = np.argsort(pair_bank, kind="stable")
    counts = np.bincount(pair_bank, minlength=NB)
    starts = np.concatenate(([0], np.cumsum(counts)))

    slot_tok = np.full((NCORES, SLOTS), -1, dtype=np.int64)
    slot_p = np.zeros((NCORES, SLOTS), dtype=np.float32)
    overflow = []
    for b in range(NB):
        c, j = divmod(b, BPC)
        s0, s1 = starts[b], starts[b + 1]
        take = min(s1 - s0, CAP)
        idx = order[s0:s0 + take]
        slot_tok[c, j * CAP: j * CAP + take] = pair_tok[idx]
        slot_p[c, j * CAP: j * CAP + take] = pair_p[idx]
        for i in order[s0 + take:s1]:
            overflow.append((int(pair_tok[i]), b, float(pair_p[i])))
    return slot_tok, slot_p, overflow


def _combine(ys, slot_tok, X, sel, prob, weights, bias, overflow):
    NT = X.shape[0]
    out = np.zeros((NT, OUT), dtype=np.float32)
    for c in range(NCORES):
        tok = slot_tok[c]
        valid = tok >= 0
        np.add.at(out, tok[valid], ys[c][valid])
    # bias term for every pair (device computes x @ W only)
    for k in range(KSEL):
        out += prob[:, k, None] * bias[sel[:, k]]
    # exact host fallback for capacity-overflow pairs (expected: none)
    for t, b, p in overflow:
        out[t] += p * (X[t] @ weights[b])
    return out


def _run_device(in_maps, trace=False, **kwargs):
    from concourse.bass_utils import run_bass_kernel_spmd
    return run_bass_kernel_spmd(_get_nc(), in_maps,
                                core_ids=list(range(NCORES)),
                                trace=trace, **kwargs)


def kernel(_trace=False, _bass_results=None, **inputs):
    tensor = np.asarray(inputs["tensor"], dtype=np.float32)
    sel = np.asarray(inputs["bank_selections"]).astype(np.int64)
    prob = np.asarray(inputs["bank_probabilities"], dtype=np.float32)
    weights = np.asarray(inputs["weights"], dtype=np.float32)
    bias = np.asarray(inputs["bias"], dtype=np.float32)

    NT = tensor.shape[0] * tensor.shape[1]
    X = tensor.reshape(NT, IN)
    sel2 = sel.reshape(NT, KSEL)
    prob2 = prob.reshape(NT, KSEL)

    slot_tok, slot_p, overflow = _route(X, sel2, prob2)

    in_maps = []
    for c in range(NCORES):
        tok = slot_tok[c]
        rows = X[np.where(tok >= 0, tok, 0)] * slot_p[c][:, None]
        xt = np.ascontiguousarray(rows.T)              # [IN, SLOTS] fp32
        wb = _bf16(weights[c * BPC:(c + 1) * BPC])     # (8, 512, 512) bf16
        in_maps.append({
            "xt": _swizzle_x(_bf16(xt)),
            "w": _swizzle_w(wb),
        })

    res = _run_device(in_maps, trace=_trace)
    if _bass_results is not None:
        _bass_results.append(res)
    # y: [128, GROUPS*OUT] f32; row r of group g is slot g*128 + r
    ys = []
    for c in range(NCORES):
        yflat = np.asarray(res.results[c]["y"]).astype(np.float32)
        ys.append(np.concatenate(
            [yflat[:, g * OUT:(g + 1) * OUT] for g in range(GROUPS)],
            axis=0))

    out = _combine(ys, slot_tok, X, sel2, prob2, weights, bias, overflow)
    return out.reshape(tensor.shape[0], tensor.shape[1], OUT)
